# revision 28
# baseline (speedup 1.0000x reference)
"""CTC loss (keras ctc_batch_cost semantics) on Trainium2, 8-core data parallel.

Bidirectional packed wavefront (per core, 64 examples on 128 partitions):
  Linear-domain CTC with per-step rescale K (p' = K*p, loss = T*log K -
  log P). The lattice is split at t = T/2: partitions 0-63 run the FORWARD
  wavefront over t in [0, 256) (columns bl_0, l_0, bl_1, ...), partitions
  64-127 run the BACKWARD wavefront over reversed time tau = 511-t
  (columns bl_48, l_47, bl_47, ...). Both recurrences have the identical
  "atilde" scan form (state = pbsh*state + lprev; x = atilde + (m-1)*lprev;
  l-scan state = (x + state)*pg), so each wavefront step is ONE [128, ~233]
  DVE tensor_tensor_scan pair + one scalar_tensor_tensor — half the serial
  elements per scan of the unidirectional version. Column i is zero for
  k < i in both directions (head pruning).

  Merge at the cut: P = sum_{s'} I_{s'} * bhat_{s'}(256), where I_{s'} is
  the forward pre-multiply inflow at t=256 (atilde/x/l slots 256) and
  bhat_{s'}(256) comes from the backward tiles (slot 255 * pb[256] for
  blanks, raw slot 256 for labels). Boundary values are packed by tiny
  scalar-engine copies during the wavefront, moved across partitions with
  one SBUF-to-SBUF DMA, and combined with two strided TT + reduce pairs.

  Data movement: the host packs y_pred as ytr2[(b*C+c)*2 + h, k] =
  bf16(K * y_pred[b, t, c]) with h=0: t=k (forward half) and h=1: t=511-k
  (reversed backward half) — pure layout/dtype/scale conditioning. Each
  label column needs ONE [128,1]-offset indirect row gather (512B rows)
  fetching the forward row for label j (top) and the reversed row for label
  47-j (bottom); gathers stream from the input underneath the wavefront.

Shapes are hardcoded for B=512, T=512, C=128, L=48 (S=97), 8 cores.
"""

import sys

if "/opt/trn_rl_repo" not in sys.path:
    sys.path.insert(0, "/opt/trn_rl_repo")

import math

import ml_dtypes
import numpy as np

import concourse.bacc as bacc
import concourse.bass as bass
import concourse.tile as tile
from concourse import mybir
from concourse.bass_utils import run_bass_kernel_spmd

NCORES = 8
B, T, C, L = 512, 512, 128, 48
BL = B // NCORES  # 64 examples per core
BLANK = C - 1
H = T // 2  # 256: the fwd/bwd cut
NS = H + 1  # 257 slots per column tile (slot k = value at time k-1)
K = 75.0  # per-step rescale; log K ~= 4.317
F32 = mybir.dt.float32
BF16 = mybir.dt.bfloat16
I32 = mybir.dt.int32
ALU = mybir.AluOpType
ACTF = mybir.ActivationFunctionType


def build_ctc_program(nc: bass.Bass):
    ytr2 = nc.dram_tensor("ytr2", [BL * C * 2, H], BF16, kind="ExternalInput").ap()
    idxd = nc.dram_tensor("idx", [2 * BL, L], I32, kind="ExternalInput").ap()
    mskd = nc.dram_tensor("msk", [2 * BL, L], F32, kind="ExternalInput").ap()
    out = nc.dram_tensor("out", [BL, 1], F32, kind="ExternalOutput").ap()

    with tile.TileContext(nc) as tc:
        _ctc_body(nc, tc, ytr2, idxd, mskd, out)
    return out


def _ctc_body(nc, tc, ytr2, idxd, mskd, out):
    P2 = 2 * BL  # 128 partitions: fwd examples | bwd examples

    with (
        tc.tile_pool(name="const", bufs=1) as cpool,
        tc.tile_pool(name="pg", bufs=48) as pgpool,
        tc.tile_pool(name="fin", bufs=1) as fpool,
    ):
        # ---- inputs ------------------------------------------------------
        # idx first: the first label-column gather is the startup long pole
        idx = cpool.tile([P2, L], I32)
        nc.sync.dma_start(out=idx[:], in_=idxd[:, :])

        # pbshc[p, k] = blank prob at step k-1 of this half; slot 0 = 1.0
        pbshc = cpool.tile([P2, NS], BF16)
        ytr4 = ytr2.rearrange("(b c h) t -> b c h t", c=C, h=2)
        nc.sync.dma_start(out=pbshc[0:BL, 1:NS], in_=ytr4[:, BLANK, 0, :])
        nc.sync.dma_start(out=pbshc[BL:P2, 1:NS], in_=ytr4[:, BLANK, 1, :])
        nc.gpsimd.memset(pbshc[:, 0:1], 1.0)

        # mc[p, j] = m - 1 in {0,-1}: x = atilde + mc*lprev (skip correction)
        mc = cpool.tile([P2, L], F32)
        nc.sync.dma_start(out=mc[:], in_=mskd[:, :])

        # touch Ln once so its table loads during startup slack
        warm = cpool.tile([BL, 1], F32)
        nc.vector.memset(warm[:], 1.0)
        nc.scalar.activation(out=warm[:], in_=warm[:], func=ACTF.Ln)

        # ---- label columns: one [128,1]-offset gather per column ---------
        pgc = []
        for j in range(L):
            pgi = pgpool.tile([P2, H], BF16, tag="pg")
            nc.gpsimd.indirect_dma_start(
                out=pgi[:],
                out_offset=None,
                in_=ytr2[:],
                in_offset=bass.IndirectOffsetOnAxis(ap=idx[:, j : j + 1], axis=0),
            )
            pgc.append(pgi)

        # ---- column storage: mega tiles (slot 256 read back at the merge)
        amega = cpool.tile([P2, (L + 1) * NS], BF16)
        lmega = cpool.tile([P2, L * NS], BF16)
        xmega = cpool.tile([P2, L * NS], BF16)
        zcol = cpool.tile([P2, NS], BF16)
        nc.gpsimd.memset(zcol[:], 0.0)
        # bwpack[64:128, i] = backward boundary values, packed in column order
        bwpack = cpool.tile([P2, 2 * L + 2], BF16)
        nc.scalar.activation(
            out=bwpack[BL:P2, 2 * L + 1 : 2 * L + 2],
            in_=pbshc[BL:P2, NS - 1 : NS], func=ACTF.Copy,
        )

        # ---- packed bidirectional wavefront ------------------------------
        lprev = zcol
        for j in range(L + 1):
            # top: fwd atilde_j[k] = a_j(k-1) + l_{j-1}(k-1)
            # bottom: bwd btilde for bl_{48-j} (same recurrence, reversed data)
            acol = amega[:, j * NS : (j + 1) * NS]
            nc.vector.tensor_tensor_scan(
                out=acol[:, j:NS], data0=pbshc[:, j:NS], data1=lprev[:, j:NS],
                initial=1.0 if j == 0 else 0.0, op0=ALU.mult, op1=ALU.add,
            )
            # bwd blank boundary: btilde_bl_{48-j}[255]
            nc.scalar.activation(
                out=bwpack[BL:P2, L - j : L - j + 1],
                in_=acol[BL:P2, NS - 2 : NS - 1], func=ACTF.Copy,
            )
            if j == L:
                break

            # x = atilde + (m-1)*lprev, one 1x DVE op (all-DVE chain beats
            # the Act-mask round trip at packed sizes)
            x = xmega[:, j * NS : (j + 1) * NS]
            nc.vector.scalar_tensor_tensor(
                out=x[:, j:NS], in0=lprev[:, j:NS], scalar=mc[:, j : j + 1],
                in1=acol[:, j:NS], op0=ALU.mult, op1=ALU.add,
            )

            # top: fwd l_j; bottom: bwd l_{47-j}
            lcol = lmega[:, j * NS : (j + 1) * NS]
            nc.vector.tensor_tensor_scan(
                out=lcol[:, j + 1 : NS], data0=x[:, j:H], data1=pgc[j][:, j:H],
                initial=0.0, op0=ALU.add, op1=ALU.mult,
            )
            # bwd label boundary: bhat_l_{47-j}(tau=255), raw slot 256
            nc.scalar.activation(
                out=bwpack[BL:P2, L + 1 + L - 1 - j : L + 1 + L - j],
                in_=lcol[BL:P2, NS - 1 : NS], func=ACTF.Copy,
            )
            lprev = lcol

        # ---- merge at the cut --------------------------------------------
        # move backward boundary vector to the forward partitions
        NB = 2 * L + 2  # 98: [btilde_bl 49 | bhat_l 48 | pb256]
        shuf = fpool.tile([BL, NB], BF16)
        nc.sync.dma_start(out=shuf[:], in_=bwpack[BL:P2, :])

        a3 = amega[:].rearrange("p (j s) -> p j s", s=NS)
        l3 = lmega[:].rearrange("p (j s) -> p j s", s=NS)
        x3 = xmega[:].rearrange("p (j s) -> p j s", s=NS)
        s3 = shuf[:].rearrange("p (j s) -> p j s", s=1)

        # blank terms: I_bl_j = atilde_j[256];  V = pb256 * btilde_bl_j[255]
        prod1 = fpool.tile([BL, L + 1], F32)
        p13 = prod1[:].rearrange("p (j s) -> p j s", s=1)
        nc.vector.tensor_tensor(
            out=p13[:], in0=a3[0:BL, :, NS - 1 : NS], in1=s3[:, 0 : L + 1, :],
            op=ALU.mult,
        )
        # label terms: I_l_j = l_j(255) + x_j[256];  V = bhat_l_j(255)
        ul = fpool.tile([BL, L], F32)
        ul3 = ul[:].rearrange("p (j s) -> p j s", s=1)
        nc.vector.tensor_tensor(
            out=ul3[:], in0=l3[0:BL, :, NS - 1 : NS], in1=x3[0:BL, :, NS - 1 : NS],
            op=ALU.add,
        )
        prod2 = fpool.tile([BL, L], F32)
        p23 = prod2[:].rearrange("p (j s) -> p j s", s=1)
        nc.vector.tensor_tensor(
            out=p23[:], in0=ul3[:], in1=s3[:, L + 1 : 2 * L + 1, :], op=ALU.mult
        )
        r1 = fpool.tile([BL, 1], F32)
        nc.vector.tensor_reduce(out=r1[:], in_=prod1[:], axis=mybir.AxisListType.X, op=ALU.add)
        r2 = fpool.tile([BL, 1], F32)
        nc.vector.tensor_reduce(out=r2[:], in_=prod2[:], axis=mybir.AxisListType.X, op=ALU.add)
        # P = pb256 * r1 + r2
        z = fpool.tile([BL, 1], F32)
        nc.vector.scalar_tensor_tensor(
            out=z[:], in0=r1[:], scalar=shuf[:, NB - 1 : NB], in1=r2[:],
            op0=ALU.mult, op1=ALU.add,
        )

        # ---- finalize: loss = T*log K - log P ----------------------------
        logz = fpool.tile([BL, 1], F32)
        nc.scalar.activation(out=logz[:], in_=z[:], func=ACTF.Ln)
        loss = fpool.tile([BL, 1], F32)
        nc.scalar.activation(
            out=loss[:], in_=logz[:], func=ACTF.Copy,
            scale=-1.0, bias=float(T * math.log(K)),
        )
        nc.sync.dma_start(out=out[:, :], in_=loss[:])


_CACHE: dict = {}


def _get_program():
    if "nc" not in _CACHE:
        nc = bacc.Bacc("TRN2", target_bir_lowering=False, debug=False)
        build_ctc_program(nc)
        nc.compile()
        _CACHE["nc"] = nc
    return _CACHE["nc"]


def kernel(y_true: np.ndarray, y_pred: np.ndarray) -> np.ndarray:
    nc = _get_program()
    lab = np.ascontiguousarray(np.asarray(y_true).astype(np.int32))  # [B, L]
    yp = np.asarray(y_pred, dtype=np.float32)  # [B, T, C]
    # input conditioning: constant K rescale folded into the bf16 quantization,
    # packed as [fwd half t=0..255 | reversed bwd half t=511..256] per class row
    yp2 = (K * yp).astype(ml_dtypes.bfloat16)
    fwd = yp2[:, :H, :].transpose(0, 2, 1)  # [B, C, 256]
    bwd = yp2[:, H:, :][:, ::-1, :].transpose(0, 2, 1)
    ytr2 = np.ascontiguousarray(np.stack([fwd, bwd], axis=2))  # [B, C, 2, 256]

    bidx = (np.arange(BL, dtype=np.int32) * C)[None, :, None]  # [1, BL, 1]
    labc = lab.reshape(NCORES, BL, L)
    idx_top = 2 * (labc + bidx)  # fwd row of label j
    idx_bot = 2 * (labc[:, :, ::-1] + bidx) + 1  # bwd row of label 47-j

    m = np.zeros((B, L), dtype=np.float32)
    m[:, 1:] = (lab[:, 1:] != lab[:, :-1]).astype(np.float32)
    mc_top = m - 1.0  # mneg_j (col 0 unused: lprev_0 = 0)
    mc_bot = np.zeros((B, L), dtype=np.float32)
    mc_bot[:, 1:] = m[:, :0:-1] - 1.0  # col j>=1: m[:, L-j] - 1
    mct = mc_top.reshape(NCORES, BL, L)
    mcb = mc_bot.reshape(NCORES, BL, L)

    in_maps = [
        {
            "ytr2": ytr2[c * BL : (c + 1) * BL].reshape(BL * C * 2, H),
            "idx": np.ascontiguousarray(
                np.concatenate([idx_top[c], idx_bot[c]], axis=0)
            ),
            "msk": np.ascontiguousarray(np.concatenate([mct[c], mcb[c]], axis=0)),
        }
        for c in range(NCORES)
    ]
    res = run_bass_kernel_spmd(nc, in_maps, list(range(NCORES)))
    return np.concatenate([res.results[c]["out"] for c in range(NCORES)], axis=0)
